# revision 25
# baseline (speedup 1.0000x reference)
"""Distributed Bass kernel: channel-LayerNorm + 4-head attention + residual.

Problem: x[2,256,4,32,32]; LN over C; qkv 1x1 conv; per-head l2norm(q,k);
softmax(8*q@kT)@v; out 1x1 conv; +residual.

Sharding: core c = b*4 + h  (b = batch, h = head). Each core:
  - gets x[b] as [C=256, N=4096], computes LN stats (mean/rstd per column)
  - computes q,k (head h) in [d, n] layout with the LN folded into the matmul
    (W' = W@diag(gamma) host-side; mean correction as a K=1 matmul; rstd
    cancels in l2norm for q,k)
  - l2-normalizes q,k via PE column-sum + PE broadcast
  - v in [n, d] layout, scaled by rstd (per-partition scalar)
  - attention: simT[j,i] = k.q blocks -> exp (ScalarE, scale=8, bf16 out)
    -> PV with v_aug=[v*rstd ; 1] as stationary -> psum [65, 512] accumulates
    over j; row 64 = softmax denominator
  - out-projection partial [256, 4096] -> ReduceScatter(4 cores of same b)
    over the N axis -> + residual slice -> out [256, 1024] per core.
Host reassembles the 8 slices.
"""

import sys

import numpy as np

if "/opt/trn_rl_repo" not in sys.path:
    sys.path.insert(0, "/opt/trn_rl_repo")

import concourse.bacc as bacc
import concourse.mybir as mybir
import concourse.tile as tile
from concourse.bass import ds, ts
from concourse.bass_utils import run_bass_kernel_spmd
from concourse.masks import make_identity

B, C, N = 2, 256, 4096
HEADS, D = 4, 64
NCORES, GROUP = 8, 4
NSLICE = N // GROUP  # 1024
SCALE, EPS, L2EPS = 8.0, 1e-5, 1e-12
F32 = mybir.dt.float32
F32R = mybir.dt.float32r
BF16 = mybir.dt.bfloat16
AF = mybir.ActivationFunctionType
NCH = 8  # 512-wide column chunks
CH = N // NCH  # 512


def _r(ap):
    """fp32 -> fp32r view: full-rate PE matmul (FP22 multiplies, fp32 accum)."""
    return ap.bitcast(F32R)


def build(with_collective=True):
    nc = bacc.Bacc(
        "TRN2", target_bir_lowering=False, debug=False, num_devices=NCORES
    )
    x_ext = nc.dram_tensor("x", [C, N], F32, kind="ExternalInput")
    wqkv_ext = nc.dram_tensor("wqkv_t", [C, 3 * D], F32, kind="ExternalInput")
    wout_ext = nc.dram_tensor("wout_t", [D, C], F32, kind="ExternalInput")
    negs_ext = nc.dram_tensor("neg_s", [1, 3 * D], F32, kind="ExternalInput")
    cones_ext = nc.dram_tensor("cones", [65, 128], F32, kind="ExternalInput")
    consts_ext = nc.dram_tensor("consts", [128, 2], F32, kind="ExternalInput")
    resid_ext = nc.dram_tensor("resid", [C, NSLICE], F32, kind="ExternalInput")
    out_ext = nc.dram_tensor("out", [C, NSLICE], F32, kind="ExternalOutput")

    with (
        nc.allow_low_precision("fp32r tags carry full fp32 bits; PE rounds"),
        tile.TileContext(nc) as tc,
        tc.tile_pool(name="singles", bufs=1) as singles,
        tc.tile_pool(name="et", bufs=12) as etp,
        tc.tile_pool(name="rcpp", bufs=4) as rcpp,
        tc.tile_pool(name="ps", bufs=4, space="PSUM") as ps,
        tc.tile_pool(name="pv", bufs=4, space="PSUM") as pvp,
        tc.tile_pool(name="dram", bufs=1, space="DRAM") as dram,
    ):
        # ---------- constants / weights ----------
        ident32 = singles.tile([32, 32], F32)
        make_identity(nc, ident32)
        # constants come from DRAM: fp32r-matmul operands need fp32r-tagged
        # producers, and DMA is the only producer codegen accepts for that
        cc_sb = singles.tile([128, 2], F32)
        nc.sync.dma_start(out=_r(cc_sb[:, :]), in_=_r(consts_ext[:, :]))
        # all-ones [65, 128]; row 64 used as a base-64 K=1 matmul operand
        ones65 = singles.tile([65, 128], F32)
        nc.sync.dma_start(out=_r(ones65[:, :]), in_=_r(cones_ext[:, :]))
        eps_b = singles.tile([32, 1], F32)
        nc.vector.memset(eps_b, EPS)
        l2eps_b = singles.tile([32, 1], F32)
        nc.vector.memset(l2eps_b, L2EPS)
        zero_col = singles.tile([128, 1], F32)
        nc.vector.memset(zero_col, 0.0)

        wqkv_sb = singles.tile([128, 2, 3 * D], F32)
        for ct in range(2):
            nc.sync.dma_start(
                out=_r(wqkv_sb[:, ct, :]), in_=_r(wqkv_ext[ds(ct * 128, 128), :])
            )
        wout_sb = singles.tile([D, C], F32)
        nc.sync.dma_start(out=_r(wout_sb[:, :]), in_=_r(wout_ext[:, :]))
        negs_sb = singles.tile([1, 3 * D], F32)
        nc.sync.dma_start(out=_r(negs_sb[:, :]), in_=_r(negs_ext[:, :]))

        x_sb = singles.tile([128, 2, N], F32)
        for ct in range(2):
            for q4 in range(4):
                nc.sync.dma_start(
                    out=_r(x_sb[:, ct, ts(q4, 1024)]),
                    in_=_r(x_ext[ds(ct * 128, 128), ts(q4, 1024)]),
                )

        # persistent attention inputs
        qn_dn = singles.tile([D, N], F32)
        kn_dn = singles.tile([D, N], F32)
        v_aug = singles.tile([128, 32, D + 1], BF16)
        rstd_col = singles.tile([128, 32], F32)
        outh_dn = singles.tile([D, N], F32)

        with (
            tc.tile_pool(name="pre", bufs=1) as pre,
            tc.tile_pool(name="prew", bufs=3) as prew,
        ):
            # ---------- LN stats: mean & E[x^2] per column ----------
            # t-form: stat_t[p, f] = stat[p*128 + f], shape [32, 128]
            mean_t = pre.tile([32, 128], F32)
            e2_t = pre.tile([32, 128], F32)
            for ch in range(NCH):
                mean_ps = ps.tile([1, CH], F32, tag="ps")
                e2_ps = ps.tile([1, CH], F32, tag="ps")
                for ct in range(2):
                    xa = x_sb[:, ct, ts(ch, CH)]
                    nc.tensor.matmul(
                        mean_ps, _r(cc_sb[:, 1:2]), _r(xa), start=(ct == 0), stop=(ct == 1)
                    )
                    x2 = prew.tile([128, CH], F32, tag="sq")
                    nc.vector.tensor_mul(_r(x2[:, :]), xa, xa)
                    nc.tensor.matmul(
                        e2_ps, _r(cc_sb[:, 1:2]), _r(x2), start=(ct == 0), stop=(ct == 1)
                    )
                ev_m = prew.tile([1, CH], F32, tag="ev")
                nc.vector.tensor_copy(ev_m, mean_ps)
                nc.sync.dma_start(out=mean_t[ds(ch * 4, 4), :], in_=ev_m)
                ev_e = prew.tile([1, CH], F32, tag="ev")
                nc.vector.tensor_copy(ev_e, e2_ps)
                nc.sync.dma_start(out=e2_t[ds(ch * 4, 4), :], in_=ev_e)

            mean_row = pre.tile([1, N], F32)
            nc.sync.dma_start(out=_r(mean_row[:, :]), in_=_r(mean_t[:, :]))

            # rstd = 1/sqrt(E[x^2] - mean^2 + eps), in t-form then transposed
            var_t = pre.tile([32, 128], F32)
            nc.vector.tensor_mul(var_t, mean_t, mean_t)
            nc.vector.tensor_sub(var_t, e2_t, var_t)
            rstd_t = pre.tile([32, 128], F32)
            nc.scalar.activation(rstd_t, var_t, AF.Sqrt, bias=eps_b)
            nc.vector.reciprocal(rstd_t, rstd_t)
            rstd_tp = ps.tile([128, 32], F32, tag="ps")
            nc.tensor.transpose(rstd_tp, rstd_t, ident32)
            nc.vector.tensor_copy(rstd_col, rstd_tp)

            # ---------- q,k raw in [d, n] layout (LN folded in) ----------
            qraw_sb = pre.tile([D, NCH, CH], F32)
            kraw_sb = pre.tile([D, NCH, CH], F32)
            qsq_t = pre.tile([32, 128], F32)
            ksq_t = pre.tile([32, 128], F32)
            for ch in range(NCH):
                for dst, sq_t, lo in ((qraw_sb, qsq_t, 0), (kraw_sb, ksq_t, D)):
                    qk_ps = ps.tile([D, CH], F32, tag="ps")
                    for ct in range(2):
                        nc.tensor.matmul(
                            qk_ps,
                            _r(wqkv_sb[:, ct, lo : lo + D]),
                            _r(x_sb[:, ct, ts(ch, CH)]),
                            start=(ct == 0),
                            stop=False,
                        )
                    nc.tensor.matmul(
                        qk_ps,
                        _r(negs_sb[:, lo : lo + D]),
                        _r(mean_row[:, ts(ch, CH)]),
                        start=False,
                        stop=True,
                    )
                    nc.scalar.copy(dst[:, ch, :], qk_ps)
                    sq = prew.tile([D, CH], F32, tag="sq")
                    nc.vector.tensor_mul(_r(sq[:, :]), dst[:, ch, :], dst[:, ch, :])
                    sq_ps = ps.tile([1, CH], F32, tag="ps")
                    nc.tensor.matmul(
                        sq_ps, _r(cc_sb[0:D, 0:1]), _r(sq), start=True, stop=True
                    )
                    ev_s = prew.tile([1, CH], F32, tag="ev")
                    nc.vector.tensor_copy(ev_s, sq_ps)
                    nc.sync.dma_start(out=sq_t[ds(ch * 4, 4), :], in_=ev_s)

            # 1/sqrt(sum q^2) rows, via t-form chains
            rq_row = pre.tile([1, N], F32)
            rk_row = pre.tile([1, N], F32)
            for sq_t, row in ((qsq_t, rq_row), (ksq_t, rk_row)):
                r_t = prew.tile([32, 128], F32, tag="rt")
                nc.scalar.activation(r_t, sq_t, AF.Sqrt, bias=l2eps_b)
                nc.vector.reciprocal(r_t, r_t)
                nc.sync.dma_start(out=_r(row[:, :]), in_=_r(r_t[:, :]))

            # normalize q,k: broadcast the row over 64 partitions via PE
            for ch in range(NCH):
                for raw, row, dn in (
                    (qraw_sb, rq_row, qn_dn),
                    (kraw_sb, rk_row, kn_dn),
                ):
                    bc_ps = ps.tile([D, CH], F32, tag="ps")
                    nc.tensor.matmul(
                        bc_ps,
                        _r(ones65[0:1, 0:D]),
                        _r(row[:, ts(ch, CH)]),
                        start=True,
                        stop=True,
                    )
                    nc.vector.tensor_mul(_r(dn[:, ts(ch, CH)]), raw[:, ch, :], bc_ps)

            # ---------- v in [n, d] layout, * rstd, plus ones column ----------
            for nb in range(32):
                v_ps = ps.tile([128, D], F32, tag="ps")
                for ct in range(2):
                    nc.tensor.matmul(
                        v_ps,
                        _r(x_sb[:, ct, ts(nb, 128)]),
                        _r(wqkv_sb[:, ct, 128:192]),
                        start=(ct == 0),
                        stop=False,
                    )
                nc.tensor.matmul(
                    v_ps,
                    _r(mean_row[:, ts(nb, 128)]),
                    _r(negs_sb[:, 128:192]),
                    start=False,
                    stop=True,
                )
                nc.vector.tensor_scalar_mul(
                    v_aug[:, nb, 0:D], v_ps, rstd_col[:, ds(nb, 1)]
                )
            nc.vector.memset(v_aug[:, :, D : D + 1], 1.0)

        # ---------- attention + projection + collective ----------
        rs_in = dram.tile([GROUP, C, NSLICE], F32)
        rs_out = dram.tile([C, NSLICE], F32)

        with (
            tc.tile_pool(name="att", bufs=3) as att,
            tc.tile_pool(name="tailp", bufs=3) as tailp,
        ):
            for hb in range(2):  # halves of the i axis (2048 each)
                pvs = []
                for _s4 in range(4):
                    pv_ps = pvp.tile([D + 1, CH], F32, tag="pv")
                    pvs.append(pv_ps)
                for j in range(32):
                    ets = []
                    for s4 in range(4):
                        i0 = hb * 2048 + s4 * CH
                        sim_ps = ps.tile([128, CH], F32, tag="ps")
                        nc.tensor.matmul(
                            sim_ps,
                            _r(kn_dn[:, ts(j, 128)]),
                            _r(qn_dn[:, ds(i0, CH)]),
                            start=True,
                            stop=True,
                        )
                        et = etp.tile([128, CH], BF16, tag="et")
                        nc.scalar.activation(
                            et, sim_ps, AF.Exp, bias=zero_col, scale=SCALE
                        )
                        ets.append(et)
                    for s4 in range(4):
                        nc.tensor.matmul(
                            pvs[s4],
                            v_aug[:, j, :],
                            ets[s4],
                            start=(j == 0),
                            stop=(j == 31),
                        )
                for s4 in range(4):
                    i0 = hb * 2048 + s4 * CH
                    rcp_t = rcpp.tile([D + 1, CH], F32, tag="rcp")
                    nc.vector.reciprocal(_r(rcp_t[D : D + 1, :]), pvs[s4][D : D + 1, :])
                    bc_ps = ps.tile([64, CH], F32, tag="ps")
                    nc.tensor.matmul(
                        bc_ps,
                        _r(ones65[D : D + 1, 0:64]),
                        _r(rcp_t[D : D + 1, :]),
                        start=True,
                        stop=True,
                    )
                    nc.vector.tensor_copy(_r(rcp_t[0:D, :]), bc_ps)
                    nc.vector.tensor_mul(
                        _r(outh_dn[:, ds(i0, CH)]), pvs[s4][0:D, :], rcp_t[0:D, :]
                    )
                    # out-projection of this 512-col chunk -> bounce buffer
                    ch = hb * 4 + s4
                    g, off = ch // 2, (ch % 2) * CH
                    for hf in range(2):
                        proj_ps = ps.tile([128, CH], F32, tag="ps")
                        nc.tensor.matmul(
                            proj_ps,
                            _r(wout_sb[:, ts(hf, 128)]),
                            _r(outh_dn[:, ds(i0, CH)]),
                            start=True,
                            stop=True,
                        )
                        proj_sb = att.tile([128, CH], F32, tag="proj")
                        nc.scalar.copy(proj_sb, proj_ps)
                        nc.sync.dma_start(
                            out=rs_in[g, ds(hf * 128, 128), ds(off, CH)], in_=proj_sb
                        )

            # ---------- reduce-scatter over the 4 cores of this batch ----------
            if with_collective:
                nc.gpsimd.collective_compute(
                    "ReduceScatter",
                    mybir.AluOpType.add,
                    replica_groups=[[0, 1, 2, 3], [4, 5, 6, 7]],
                    ins=[rs_in.opt()],
                    outs=[rs_out.opt()],
                )
            else:
                nc.sync.dma_start(out=rs_out[:, :], in_=rs_in[0, :, :])

            # ---------- + residual, write out ----------
            for ct in range(2):
                res_sb = tailp.tile([128, NSLICE], F32, tag="tail")
                nc.sync.dma_start(out=res_sb, in_=resid_ext[ds(ct * 128, 128), :])
                rs_sb = tailp.tile([128, NSLICE], F32, tag="tail")
                nc.sync.dma_start(out=rs_sb, in_=rs_out[ds(ct * 128, 128), :])
                osum = tailp.tile([128, NSLICE], F32, tag="tail")
                nc.vector.tensor_add(osum, rs_sb, res_sb)
                nc.sync.dma_start(out=out_ext[ds(ct * 128, 128), :], in_=osum)

    nc.finalize()
    return nc


_CACHE = {}


def _get_nc():
    if "nc" not in _CACHE:
        _CACHE["nc"] = build()
    return _CACHE["nc"]


def _make_in_maps(x, gamma, w_qkv, w_out):
    x = np.asarray(x, dtype=np.float32).reshape(B, C, N)
    gamma_c = np.asarray(gamma, dtype=np.float32).reshape(C)
    w_qkv = np.asarray(w_qkv, dtype=np.float32)
    w_out = np.asarray(w_out, dtype=np.float32)

    wp = w_qkv * gamma_c[None, :]  # fold gamma: W' = W @ diag(gamma)
    in_maps = []
    for c in range(NCORES):
        b, h = c // GROUP, c % GROUP
        rows = np.concatenate(
            [
                wp[h * D : (h + 1) * D],  # q rows
                wp[C + h * D : C + (h + 1) * D],  # k rows
                wp[2 * C + h * D : 2 * C + (h + 1) * D],  # v rows
            ],
            axis=0,
        )  # [192, 256]
        neg_s = -rows.sum(axis=1)[None, :]  # [1, 192]
        wout_t = np.ascontiguousarray(w_out[:, h * D : (h + 1) * D].T)  # [64, 256]
        in_maps.append(
            {
                "x": np.ascontiguousarray(x[b]),
                "wqkv_t": np.ascontiguousarray(rows.T),
                "wout_t": wout_t,
                "neg_s": np.ascontiguousarray(neg_s),
                "cones": np.ones((65, 128), dtype=np.float32),
                "consts": np.ascontiguousarray(
                    np.stack(
                        [np.ones(128, np.float32), np.full(128, 1.0 / C, np.float32)],
                        axis=1,
                    )
                ),
                "resid": np.ascontiguousarray(x[b][:, h * NSLICE : (h + 1) * NSLICE]),
            }
        )
    return in_maps


def _assemble(results):
    out = np.zeros((B, C, N), dtype=np.float32)
    for c in range(NCORES):
        b, h = c // GROUP, c % GROUP
        out[b][:, h * NSLICE : (h + 1) * NSLICE] = results[c]["out"]
    return out.reshape(B, C, 4, 32, 32)


def run(inputs, trace=False, **kw):
    nc = _get_nc()
    in_maps = _make_in_maps(**inputs)
    res = run_bass_kernel_spmd(
        nc, in_maps, core_ids=list(range(NCORES)), trace=trace, **kw
    )
    return _assemble(res.results), res


def kernel(x, gamma, w_qkv, w_out):
    out, _ = run(dict(x=x, gamma=gamma, w_qkv=w_qkv, w_out=w_out))
    return out


# revision 26
# speedup vs baseline: 1.0829x; 1.0829x over previous
"""Distributed Bass kernel: channel-LayerNorm + 4-head attention + residual.

Problem: x[2,256,4,32,32]; LN over C; qkv 1x1 conv; per-head l2norm(q,k);
softmax(8*q@kT)@v; out 1x1 conv; +residual.

Sharding: core c = b*4 + h  (b = batch, h = head). Each core:
  - gets x[b] as [C=256, N=4096], computes LN stats (mean/rstd per column)
  - computes q,k (head h) in [d, n] layout with the LN folded into the matmul
    (W' = W@diag(gamma) host-side; mean correction as a K=1 matmul; rstd
    cancels in l2norm for q,k)
  - l2-normalizes q,k via PE column-sum + PE broadcast
  - v in [n, d] layout, scaled by rstd (per-partition scalar)
  - attention: simT[j,i] = k.q blocks -> exp (ScalarE, scale=8, bf16 out)
    -> PV with v_aug=[v*rstd ; 1] as stationary -> psum [65, 512] accumulates
    over j; row 64 = softmax denominator
  - out-projection partial [256, 4096] -> ReduceScatter(4 cores of same b)
    over the N axis -> + residual slice -> out [256, 1024] per core.
Host reassembles the 8 slices.
"""

import sys

import numpy as np

if "/opt/trn_rl_repo" not in sys.path:
    sys.path.insert(0, "/opt/trn_rl_repo")

import concourse.bacc as bacc
import concourse.mybir as mybir
import concourse.tile as tile
from concourse.bass import ds, ts
from concourse.bass_utils import run_bass_kernel_spmd
from concourse.masks import make_identity

B, C, N = 2, 256, 4096
HEADS, D = 4, 64
NCORES, GROUP = 8, 4
NSLICE = N // GROUP  # 1024
SCALE, EPS, L2EPS = 8.0, 1e-5, 1e-12
F32 = mybir.dt.float32
F32R = mybir.dt.float32r
BF16 = mybir.dt.bfloat16
AF = mybir.ActivationFunctionType
NCH = 8  # 512-wide column chunks
CH = N // NCH  # 512


def _r(ap):
    """fp32 -> fp32r view: full-rate PE matmul (FP22 multiplies, fp32 accum)."""
    return ap.bitcast(F32R)


def build(with_collective=True):
    nc = bacc.Bacc(
        "TRN2", target_bir_lowering=False, debug=False, num_devices=NCORES
    )
    x_ext = nc.dram_tensor("x", [C, N], F32, kind="ExternalInput")
    wqkv_ext = nc.dram_tensor("wqkv_t", [C, 3 * D], F32, kind="ExternalInput")
    wout_ext = nc.dram_tensor("wout_t", [D, C], F32, kind="ExternalInput")
    negs_ext = nc.dram_tensor("neg_s", [1, 3 * D], F32, kind="ExternalInput")
    cones_ext = nc.dram_tensor("cones", [65, 128], F32, kind="ExternalInput")
    consts_ext = nc.dram_tensor("consts", [128, 2], F32, kind="ExternalInput")
    resid_ext = nc.dram_tensor("resid", [C, NSLICE], F32, kind="ExternalInput")
    out_ext = nc.dram_tensor("out", [C, NSLICE], F32, kind="ExternalOutput")

    with (
        nc.allow_low_precision("fp32r tags carry full fp32 bits; PE rounds"),
        tile.TileContext(nc) as tc,
        tc.tile_pool(name="singles", bufs=1) as singles,
        tc.tile_pool(name="et", bufs=12) as etp,
        tc.tile_pool(name="rcpp", bufs=4) as rcpp,
        tc.tile_pool(name="ps", bufs=4, space="PSUM") as ps,
        tc.tile_pool(name="pv", bufs=4, space="PSUM") as pvp,
        tc.tile_pool(name="dram", bufs=1, space="DRAM") as dram,
    ):
        # ---------- constants / weights ----------
        ident32 = singles.tile([32, 32], F32)
        make_identity(nc, ident32)
        # constants come from DRAM: fp32r-matmul operands need fp32r-tagged
        # producers, and DMA is the only producer codegen accepts for that
        cc_sb = singles.tile([128, 2], F32)
        nc.sync.dma_start(out=_r(cc_sb[:, :]), in_=_r(consts_ext[:, :]))
        # all-ones [65, 128]; row 64 used as a base-64 K=1 matmul operand
        ones65 = singles.tile([65, 128], F32)
        nc.sync.dma_start(out=_r(ones65[:, :]), in_=_r(cones_ext[:, :]))
        eps_b = singles.tile([32, 1], F32)
        nc.vector.memset(eps_b, EPS)
        l2eps_b = singles.tile([32, 1], F32)
        nc.vector.memset(l2eps_b, L2EPS)
        zero_col = singles.tile([128, 1], F32)
        nc.vector.memset(zero_col, 0.0)

        wqkv_sb = singles.tile([128, 2, 3 * D], F32)
        for ct in range(2):
            nc.sync.dma_start(
                out=_r(wqkv_sb[:, ct, :]), in_=_r(wqkv_ext[ds(ct * 128, 128), :])
            )
        wout_sb = singles.tile([D, C], F32)
        nc.sync.dma_start(out=_r(wout_sb[:, :]), in_=_r(wout_ext[:, :]))
        negs_sb = singles.tile([1, 3 * D], F32)
        nc.sync.dma_start(out=_r(negs_sb[:, :]), in_=_r(negs_ext[:, :]))

        x_sb = singles.tile([128, 2, N], F32)
        for ct in range(2):
            for q4 in range(4):
                nc.sync.dma_start(
                    out=_r(x_sb[:, ct, ts(q4, 1024)]),
                    in_=_r(x_ext[ds(ct * 128, 128), ts(q4, 1024)]),
                )

        # persistent attention inputs
        qn_dn = singles.tile([D, N], BF16)
        kn_dn = singles.tile([D, N], BF16)
        # padded to 128 weight columns so LDWEIGHTS takes the fast path;
        # cols 65:128 are zero
        v_aug = singles.tile([128, 32, 128], BF16)
        rstd_col = singles.tile([128, 32], F32)
        outh_dn = singles.tile([D, N], F32)

        with (
            tc.tile_pool(name="pre", bufs=1) as pre,
            tc.tile_pool(name="prew", bufs=3) as prew,
        ):
            # ---------- LN stats: mean & E[x^2] per column ----------
            # t-form: stat_t[p, f] = stat[p*128 + f], shape [32, 128]
            mean_t = pre.tile([32, 128], F32)
            e2_t = pre.tile([32, 128], F32)
            for ch in range(NCH):
                mean_ps = ps.tile([1, CH], F32, tag="ps")
                e2_ps = ps.tile([1, CH], F32, tag="ps")
                for ct in range(2):
                    xa = x_sb[:, ct, ts(ch, CH)]
                    nc.tensor.matmul(
                        mean_ps, _r(cc_sb[:, 1:2]), _r(xa), start=(ct == 0), stop=(ct == 1)
                    )
                    x2 = prew.tile([128, CH], F32, tag="sq")
                    nc.vector.tensor_mul(_r(x2[:, :]), xa, xa)
                    nc.tensor.matmul(
                        e2_ps, _r(cc_sb[:, 1:2]), _r(x2), start=(ct == 0), stop=(ct == 1)
                    )
                ev_m = prew.tile([1, CH], F32, tag="ev")
                nc.vector.tensor_copy(ev_m, mean_ps)
                nc.sync.dma_start(out=mean_t[ds(ch * 4, 4), :], in_=ev_m)
                ev_e = prew.tile([1, CH], F32, tag="ev")
                nc.vector.tensor_copy(ev_e, e2_ps)
                nc.sync.dma_start(out=e2_t[ds(ch * 4, 4), :], in_=ev_e)

            mean_row = pre.tile([1, N], F32)
            nc.sync.dma_start(out=_r(mean_row[:, :]), in_=_r(mean_t[:, :]))

            # rstd = 1/sqrt(E[x^2] - mean^2 + eps), in t-form then transposed
            var_t = pre.tile([32, 128], F32)
            nc.vector.tensor_mul(var_t, mean_t, mean_t)
            nc.vector.tensor_sub(var_t, e2_t, var_t)
            rstd_t = pre.tile([32, 128], F32)
            nc.scalar.activation(rstd_t, var_t, AF.Sqrt, bias=eps_b)
            nc.vector.reciprocal(rstd_t, rstd_t)
            rstd_tp = ps.tile([128, 32], F32, tag="ps")
            nc.tensor.transpose(rstd_tp, rstd_t, ident32)
            nc.vector.tensor_copy(rstd_col, rstd_tp)

            # ---------- q,k raw in [d, n] layout (LN folded in) ----------
            qraw_sb = pre.tile([D, NCH, CH], F32)
            kraw_sb = pre.tile([D, NCH, CH], F32)
            qsq_t = pre.tile([32, 128], F32)
            ksq_t = pre.tile([32, 128], F32)
            for ch in range(NCH):
                for dst, sq_t, lo in ((qraw_sb, qsq_t, 0), (kraw_sb, ksq_t, D)):
                    qk_ps = ps.tile([D, CH], F32, tag="ps")
                    for ct in range(2):
                        nc.tensor.matmul(
                            qk_ps,
                            _r(wqkv_sb[:, ct, lo : lo + D]),
                            _r(x_sb[:, ct, ts(ch, CH)]),
                            start=(ct == 0),
                            stop=False,
                        )
                    nc.tensor.matmul(
                        qk_ps,
                        _r(negs_sb[:, lo : lo + D]),
                        _r(mean_row[:, ts(ch, CH)]),
                        start=False,
                        stop=True,
                    )
                    nc.scalar.copy(dst[:, ch, :], qk_ps)
                    sq = prew.tile([D, CH], F32, tag="sq")
                    nc.vector.tensor_mul(_r(sq[:, :]), dst[:, ch, :], dst[:, ch, :])
                    sq_ps = ps.tile([1, CH], F32, tag="ps")
                    nc.tensor.matmul(
                        sq_ps, _r(cc_sb[0:D, 0:1]), _r(sq), start=True, stop=True
                    )
                    ev_s = prew.tile([1, CH], F32, tag="ev")
                    nc.vector.tensor_copy(ev_s, sq_ps)
                    nc.sync.dma_start(out=sq_t[ds(ch * 4, 4), :], in_=ev_s)

            # 1/sqrt(sum q^2) rows, via t-form chains
            rq_row = pre.tile([1, N], F32)
            rk_row = pre.tile([1, N], F32)
            for sq_t, row in ((qsq_t, rq_row), (ksq_t, rk_row)):
                r_t = prew.tile([32, 128], F32, tag="rt")
                nc.scalar.activation(r_t, sq_t, AF.Sqrt, bias=l2eps_b)
                nc.vector.reciprocal(r_t, r_t)
                nc.sync.dma_start(out=_r(row[:, :]), in_=_r(r_t[:, :]))

            # normalize q,k: broadcast the row over 64 partitions via PE
            for ch in range(NCH):
                for raw, row, dn in (
                    (qraw_sb, rq_row, qn_dn),
                    (kraw_sb, rk_row, kn_dn),
                ):
                    bc_ps = ps.tile([D, CH], F32, tag="ps")
                    nc.tensor.matmul(
                        bc_ps,
                        _r(ones65[0:1, 0:D]),
                        _r(row[:, ts(ch, CH)]),
                        start=True,
                        stop=True,
                    )
                    nc.vector.tensor_mul(dn[:, ts(ch, CH)], raw[:, ch, :], bc_ps)

            # ---------- v in [n, d] layout, * rstd, plus ones column ----------
            for nb in range(32):
                v_ps = ps.tile([128, D], F32, tag="ps")
                for ct in range(2):
                    nc.tensor.matmul(
                        v_ps,
                        _r(x_sb[:, ct, ts(nb, 128)]),
                        _r(wqkv_sb[:, ct, 128:192]),
                        start=(ct == 0),
                        stop=False,
                    )
                nc.tensor.matmul(
                    v_ps,
                    _r(mean_row[:, ts(nb, 128)]),
                    _r(negs_sb[:, 128:192]),
                    start=False,
                    stop=True,
                )
                nc.vector.tensor_scalar_mul(
                    v_aug[:, nb, 0:D], v_ps, rstd_col[:, ds(nb, 1)]
                )
            nc.vector.memset(v_aug[:, :, D : D + 1], 1.0)
            nc.vector.memset(v_aug[:, :, D + 1 : 128], 0.0)

        # ---------- attention + projection + collective ----------
        rs_in = dram.tile([GROUP, C, NSLICE], F32)
        rs_out = dram.tile([C, NSLICE], F32)

        with (
            tc.tile_pool(name="att", bufs=3) as att,
            tc.tile_pool(name="tailp", bufs=3) as tailp,
        ):
            for hb in range(2):  # halves of the i axis (2048 each)
                pvs = []
                for _s4 in range(4):
                    pv_ps = pvp.tile([128, CH], F32, tag="pv")
                    pvs.append(pv_ps)
                for j in range(32):
                    ets = []
                    for s4 in range(4):
                        i0 = hb * 2048 + s4 * CH
                        sim_ps = ps.tile([128, CH], F32, tag="ps")
                        nc.tensor.matmul(
                            sim_ps,
                            kn_dn[:, ts(j, 128)],
                            qn_dn[:, ds(i0, CH)],
                            start=True,
                            stop=True,
                        )
                        et = etp.tile([128, CH], BF16, tag="et")
                        nc.scalar.activation(
                            et, sim_ps, AF.Exp, bias=zero_col, scale=SCALE
                        )
                        ets.append(et)
                    for s4 in range(4):
                        nc.tensor.matmul(
                            pvs[s4],
                            v_aug[:, j, :],
                            ets[s4],
                            start=(j == 0),
                            stop=(j == 31),
                        )
                for s4 in range(4):
                    i0 = hb * 2048 + s4 * CH
                    rcp_t = rcpp.tile([D + 1, CH], F32, tag="rcp")
                    nc.vector.reciprocal(_r(rcp_t[D : D + 1, :]), pvs[s4][D : D + 1, :])
                    bc_ps = ps.tile([64, CH], F32, tag="ps")
                    nc.tensor.matmul(
                        bc_ps,
                        _r(ones65[D : D + 1, 0:64]),
                        _r(rcp_t[D : D + 1, :]),
                        start=True,
                        stop=True,
                    )
                    nc.vector.tensor_copy(_r(rcp_t[0:D, :]), bc_ps)
                    nc.vector.tensor_mul(
                        _r(outh_dn[:, ds(i0, CH)]), pvs[s4][0:D, :], rcp_t[0:D, :]
                    )
                    # out-projection of this 512-col chunk -> bounce buffer
                    ch = hb * 4 + s4
                    g, off = ch // 2, (ch % 2) * CH
                    for hf in range(2):
                        proj_ps = ps.tile([128, CH], F32, tag="ps")
                        nc.tensor.matmul(
                            proj_ps,
                            _r(wout_sb[:, ts(hf, 128)]),
                            _r(outh_dn[:, ds(i0, CH)]),
                            start=True,
                            stop=True,
                        )
                        proj_sb = att.tile([128, CH], F32, tag="proj")
                        nc.scalar.copy(proj_sb, proj_ps)
                        nc.sync.dma_start(
                            out=rs_in[g, ds(hf * 128, 128), ds(off, CH)], in_=proj_sb
                        )

            # ---------- reduce-scatter over the 4 cores of this batch ----------
            if with_collective:
                nc.gpsimd.collective_compute(
                    "ReduceScatter",
                    mybir.AluOpType.add,
                    replica_groups=[[0, 1, 2, 3], [4, 5, 6, 7]],
                    ins=[rs_in.opt()],
                    outs=[rs_out.opt()],
                )
            else:
                nc.sync.dma_start(out=rs_out[:, :], in_=rs_in[0, :, :])

            # ---------- + residual, write out ----------
            for ct in range(2):
                res_sb = tailp.tile([128, NSLICE], F32, tag="tail")
                nc.sync.dma_start(out=res_sb, in_=resid_ext[ds(ct * 128, 128), :])
                rs_sb = tailp.tile([128, NSLICE], F32, tag="tail")
                nc.sync.dma_start(out=rs_sb, in_=rs_out[ds(ct * 128, 128), :])
                osum = tailp.tile([128, NSLICE], F32, tag="tail")
                nc.vector.tensor_add(osum, rs_sb, res_sb)
                nc.sync.dma_start(out=out_ext[ds(ct * 128, 128), :], in_=osum)

    nc.finalize()
    return nc


_CACHE = {}


def _get_nc():
    if "nc" not in _CACHE:
        _CACHE["nc"] = build()
    return _CACHE["nc"]


def _make_in_maps(x, gamma, w_qkv, w_out):
    x = np.asarray(x, dtype=np.float32).reshape(B, C, N)
    gamma_c = np.asarray(gamma, dtype=np.float32).reshape(C)
    w_qkv = np.asarray(w_qkv, dtype=np.float32)
    w_out = np.asarray(w_out, dtype=np.float32)

    wp = w_qkv * gamma_c[None, :]  # fold gamma: W' = W @ diag(gamma)
    in_maps = []
    for c in range(NCORES):
        b, h = c // GROUP, c % GROUP
        rows = np.concatenate(
            [
                wp[h * D : (h + 1) * D],  # q rows
                wp[C + h * D : C + (h + 1) * D],  # k rows
                wp[2 * C + h * D : 2 * C + (h + 1) * D],  # v rows
            ],
            axis=0,
        )  # [192, 256]
        neg_s = -rows.sum(axis=1)[None, :]  # [1, 192]
        wout_t = np.ascontiguousarray(w_out[:, h * D : (h + 1) * D].T)  # [64, 256]
        in_maps.append(
            {
                "x": np.ascontiguousarray(x[b]),
                "wqkv_t": np.ascontiguousarray(rows.T),
                "wout_t": wout_t,
                "neg_s": np.ascontiguousarray(neg_s),
                "cones": np.ones((65, 128), dtype=np.float32),
                "consts": np.ascontiguousarray(
                    np.stack(
                        [np.ones(128, np.float32), np.full(128, 1.0 / C, np.float32)],
                        axis=1,
                    )
                ),
                "resid": np.ascontiguousarray(x[b][:, h * NSLICE : (h + 1) * NSLICE]),
            }
        )
    return in_maps


def _assemble(results):
    out = np.zeros((B, C, N), dtype=np.float32)
    for c in range(NCORES):
        b, h = c // GROUP, c % GROUP
        out[b][:, h * NSLICE : (h + 1) * NSLICE] = results[c]["out"]
    return out.reshape(B, C, 4, 32, 32)


def run(inputs, trace=False, **kw):
    nc = _get_nc()
    in_maps = _make_in_maps(**inputs)
    res = run_bass_kernel_spmd(
        nc, in_maps, core_ids=list(range(NCORES)), trace=trace, **kw
    )
    return _assemble(res.results), res


def kernel(x, gamma, w_qkv, w_out):
    out, _ = run(dict(x=x, gamma=gamma, w_qkv=w_qkv, w_out=w_out))
    return out
